# revision 14
# baseline (speedup 1.0000x reference)
"""Inverted-dropout kernel for Trainium2, distributed over 8 NeuronCores.

Computes out = where(mask, x * 2.0, 0) for x:(64,2048,4,7,7) f32 and
mask:(64,2048,4,7,7) bool.  Pure elementwise: shard along batch (8 per core).
Each core streams its shard HBM->SBUF in [128, 3584] f32 tiles, applies one
fused DVE TensorTensor op (the host pre-folds the 1/(1-p)=2.0 dropout scale
into the byte mask, so the op is just x * mask2 with mask2 in {0,2} read
directly as uint8), and streams the result back.  Loads/stores alternate
between the two HWDGE rings (SP / ACT) to balance DMA bytes across rings.

Measured ~68-70 us per core steady-state (~28.9 MB HBM traffic per core at
~420 GB/s — at the practical HBM/fabric ceiling; TimelineSim predicts 84 us).
"""

import sys

import numpy as np

try:
    import concourse.bacc as bacc
except ImportError:  # grading env without the default sys.path site config
    for p in ("/root/.axon_site/_ro/trn_rl_repo", "/opt/trn_rl_repo"):
        if p not in sys.path:
            sys.path.append(p)
    import concourse.bacc as bacc

import concourse.mybir as mybir
from concourse.tile import TileContext

# Full problem shape (hardcoded per harness contract).
B, C, FM, H, W = 64, 2048, 4, 7, 7
N_CORES = 8
B_PER_CORE = B // N_CORES                       # 8
ELEMS_PER_CORE = B_PER_CORE * C * FM * H * W    # 3,211,264 = 7 * 128 * 3584

P = 128          # SBUF partitions
F = 3584         # free-dim elems per tile  (128*3584*4B = 1.75 MiB per x DMA)
NT = ELEMS_PER_CORE // (P * F)                  # 7 tiles
assert NT * P * F == ELEMS_PER_CORE

SCALE = 2.0      # 1 / (1 - p_drop), p_drop = 0.5
BUFS = 6


def build_nc(nt=NT, f=F, bufs=None, repeat=1):
    """Build the per-core SPMD module (phase-structured).

    - Bacc (not bare Bass): Bacc.compile() legalizes sync waits down to the
      TRN2 1-wait-per-instruction limit — walrus rejects the module otherwise.
    - Phase structure: ALL loads are enqueued first, then the in-place DVE
      muls, then ALL stores.  HWDGE rings drain FIFO per issuing engine, so
      this gives natural read/write phase separation at the HBM: measured
      pure-read ~457 GB/s and pure-write ~554 GB/s vs only ~430 GB/s for
      packet-interleaved mixed traffic.  The whole per-core shard stays
      resident in SBUF (in-place DVE output keeps it at 5*F*nt = 125 KB
      per partition).  Consistently matched or beat the interleaved
      per-tile pipeline on HW (66-67 us vs 68-70 us quiet-window).
    - Loads and stores still alternate between the two HWDGE rings (SP via
      nc.sync, ACT via nc.scalar) to balance ring bytes.
    - repeat>1 unrolls the whole body R times inside one NEFF (idempotent
      rewrites of the same output), used only for launch-overhead-free timing
      via (T(R2)-T(R1))/(R2-R1).
    """
    nc = bacc.Bacc()
    x = nc.declare_dram_parameter("x", [nt, P, f], mybir.dt.float32, isOutput=False)
    m = nc.declare_dram_parameter("mask", [nt, P, f], mybir.dt.uint8, isOutput=False)
    o = nc.declare_dram_parameter("out", [nt, P, f], mybir.dt.float32, isOutput=True)
    with TileContext(nc) as tc:
        with tc.tile_pool(name="sbuf", bufs=nt) as pool:
            for _ in range(repeat):
                xts, mts = [], []
                for i in range(nt):
                    load_eng = nc.sync if i % 2 == 0 else nc.scalar
                    xt = pool.tile([P, f], mybir.dt.float32)
                    load_eng.dma_start(out=xt[:], in_=x[i])
                    mt = pool.tile([P, f], mybir.dt.uint8)
                    load_eng.dma_start(out=mt[:], in_=m[i])
                    xts.append(xt)
                    mts.append(mt)
                for i in range(nt):
                    # mask arrives pre-scaled to {0, 2} (host folds the
                    # 1/(1-p) dropout scale into the byte mask), so one
                    # in-place TensorTensor op computes the whole dropout.
                    nc.vector.tensor_mul(out=xts[i][:], in0=xts[i][:], in1=mts[i][:])
                for i in range(nt):
                    store_eng = nc.scalar if i % 2 == 0 else nc.sync
                    store_eng.dma_start(out=o[i], in_=xts[i][:])
    nc.compile()
    return nc


def _build_runner(nc, n_cores):
    """Compile the SPMD module into a reusable shard_map-jitted callable.

    Same machinery as bass2jax.run_bass_via_pjrt, but the jitted function is
    built once and cached so repeated kernel() calls skip XLA re-tracing.
    Output-buffer donation is dropped: this kernel writes every output
    element, so zero-initialized outputs are unnecessary.
    """
    import jax
    from jax.sharding import Mesh, PartitionSpec, NamedSharding
    from jax.experimental.shard_map import shard_map
    from concourse.bass2jax import (
        _bass_exec_p,
        install_neuronx_cc_hook,
        partition_id_tensor,
    )

    install_neuronx_cc_hook()
    partition_name = nc.partition_id_tensor.name if nc.partition_id_tensor else None

    in_names, out_names, out_avals = [], [], []
    for alloc in nc.m.functions[0].allocations:
        if not isinstance(alloc, mybir.MemoryLocationSet):
            continue
        name = alloc.memorylocations[0].name
        if alloc.kind == "ExternalInput":
            if name != partition_name:
                in_names.append(name)
        elif alloc.kind == "ExternalOutput":
            out_names.append(name)
            out_avals.append(
                jax.core.ShapedArray(
                    tuple(alloc.tensor_shape), mybir.dt.np(alloc.dtype)
                )
            )
    n_params = len(in_names)
    all_in_names = list(in_names) + list(out_names)
    if partition_name is not None:
        all_in_names.append(partition_name)

    def _body(*args):
        operands = list(args)
        if partition_name is not None:
            operands.append(partition_id_tensor())
        outs = _bass_exec_p.bind(
            *operands,
            out_avals=tuple(out_avals),
            in_names=tuple(all_in_names),
            out_names=tuple(out_names),
            lowering_input_output_aliases=(),
            sim_require_finite=True,
            sim_require_nnan=True,
            nc=nc,
        )
        return tuple(outs)

    devices = jax.devices()[:n_cores]
    assert len(devices) == n_cores, (
        f"need {n_cores} devices, have {len(jax.devices())}"
    )
    mesh = Mesh(np.asarray(devices), ("core",))
    in_specs = (PartitionSpec("core"),) * (n_params + len(out_names))
    out_specs = (PartitionSpec("core"),) * len(out_names)
    fn = jax.jit(
        shard_map(
            _body, mesh=mesh, in_specs=in_specs, out_specs=out_specs,
            check_rep=False,
        ),
        keep_unused=True,
    )
    sharding = NamedSharding(mesh, PartitionSpec("core"))
    zeros = [
        np.zeros((n_cores * a.shape[0], *a.shape[1:]), a.dtype) for a in out_avals
    ]
    return fn, sharding, in_names, out_avals, zeros


_CACHE = {}


def _get_runner():
    if "runner" not in _CACHE:
        nc = build_nc()
        _CACHE["runner"] = _build_runner(nc, N_CORES)
    return _CACHE["runner"]


def kernel(x: np.ndarray, mask: np.ndarray, **_) -> np.ndarray:
    import jax

    x = np.ascontiguousarray(np.asarray(x), dtype=np.float32)
    mask = np.asarray(mask)
    if mask.dtype.itemsize != 1:
        mask = mask.astype(np.bool_)
    mask = np.ascontiguousarray(mask)
    assert x.shape == (B, C, FM, H, W), x.shape
    assert mask.shape == (B, C, FM, H, W), mask.shape

    fn, sharding, in_names, out_avals, zeros = _get_runner()
    # Batch-sharding == row-blocks of the flat [N_CORES*NT, P, F] view, so
    # the global concatenated operand is just a zero-copy reshape of the
    # full input.  {0,1} bool bytes -> {0,2} u8 folds the dropout scale
    # into the mask (one cheap byte-op pass).
    global_in = {
        "x": x.reshape(N_CORES * NT, P, F),
        "mask": (mask.view(np.uint8) << 1).reshape(N_CORES * NT, P, F),
    }
    if "zeros_dev" not in _CACHE:
        # Output buffers are fully overwritten by the kernel; stage the
        # operand once and reuse it across calls (not donated).
        _CACHE["zeros_dev"] = [jax.device_put(z, sharding) for z in zeros]
    args = [jax.device_put(global_in[n], sharding) for n in in_names]
    args += _CACHE["zeros_dev"]
    out = jax.block_until_ready(fn(*args))
    return np.asarray(out[0]).reshape(B, C, FM, H, W)


# revision 16
# speedup vs baseline: 1.0579x; 1.0579x over previous
"""Inverted-dropout kernel for Trainium2, distributed over 8 NeuronCores.

Computes out = where(mask, x * 2.0, 0) for x:(64,2048,4,7,7) f32 and
mask:(64,2048,4,7,7) bool.  Pure elementwise: shard along batch (8 per core).

Design (each refinement HW-measured):
- Host folds the 1/(1-p)=2.0 dropout scale into the byte mask ({0,1} bool ->
  {0,2} u8), so the device does ONE DVE TensorTensor op per tile, reading the
  mask directly as uint8 — no cast op, mask HBM traffic stays 1 B/elem.
- Phase structure: ALL loads enqueue first, then in-place DVE muls, then ALL
  stores.  HWDGE rings drain FIFO per issuing engine, so reads and writes
  phase-separate at the HBM; measured pure-read ~457 GB/s and pure-write
  ~554 GB/s vs only ~430 GB/s for packet-interleaved mixed traffic.
- Ramp tiling (6x3584 + 4x896): small tiles last shrink the exposed
  final-tile DVE latency between the load and store phases (~3.8us -> ~1us).
- 1D flat DRAM layout: every tile is one fully contiguous chunk viewed as
  [128, w] — max-efficiency DMA descriptors and zero-copy host reshapes.
- Whole per-core shard stays SBUF-resident (in-place DVE output => ~125 KB
  of the 192 KB per partition).
- Loads/stores alternate between the two HWDGE rings (SP / ACT).

Measured ~58-61 us per core quiet-window (~28.9 MB HBM traffic per core,
~480-500 GB/s aggregate); ~100 us under neighbor-tenant HBM contention.
"""

import sys

import numpy as np

try:
    import concourse.bacc as bacc
except ImportError:  # grading env without the default sys.path site config
    for p in ("/root/.axon_site/_ro/trn_rl_repo", "/opt/trn_rl_repo"):
        if p not in sys.path:
            sys.path.append(p)
    import concourse.bacc as bacc

import concourse.mybir as mybir
from concourse.tile import TileContext

# Full problem shape (hardcoded per harness contract).
B, C, FM, H, W = 64, 2048, 4, 7, 7
N_CORES = 8
B_PER_CORE = B // N_CORES                       # 8
ELEMS_PER_CORE = B_PER_CORE * C * FM * H * W    # 3,211,264 = 128 * 25088

P = 128                                         # SBUF partitions
TOTAL_F = ELEMS_PER_CORE // P                   # 25088 free-dim elems/partition
SIZES = [3584] * 6 + [896] * 4                  # ramp: small tiles last
assert sum(SIZES) == TOTAL_F

SCALE = 2.0      # 1 / (1 - p_drop), p_drop = 0.5


def build_nc(sizes=None, repeat=1, rev_store=False):
    """Build the per-core SPMD module (phase-structured, ramp-tiled).

    Bacc (not bare Bass): Bacc.compile() legalizes sync waits down to the
    TRN2 1-wait-per-instruction limit — walrus rejects the module otherwise.

    repeat>1 unrolls the whole body R times inside one NEFF (idempotent
    rewrites of the same output), used only for launch-overhead-free timing
    via (T(R2)-T(R1))/(R2-R1).  rev_store reverses per-repeat store order so
    cross-repeat WAR chains approximate clean serial load/store phases
    (timing only; production single-shot uses forward order).
    """
    sizes = sizes or SIZES
    n = P * sum(sizes)
    nc = bacc.Bacc()
    x = nc.declare_dram_parameter("x", [n], mybir.dt.float32, isOutput=False)
    m = nc.declare_dram_parameter("mask", [n], mybir.dt.uint8, isOutput=False)
    o = nc.declare_dram_parameter("out", [n], mybir.dt.float32, isOutput=True)
    offs = np.cumsum([0] + list(sizes))[:-1]

    def sl(t, a, w):
        # contiguous flat chunk [128*a, 128*(a+w)) viewed as [128, w]
        return t[P * a: P * (a + w)].rearrange("(p w) -> p w", p=P)

    with TileContext(nc) as tc:
        with tc.tile_pool(name="sbuf", bufs=1) as pool:
            for _ in range(repeat):
                xts, mts = [], []
                for i, (a, w) in enumerate(zip(offs, sizes)):
                    load_eng = nc.sync if i % 2 == 0 else nc.scalar
                    xt = pool.tile([P, w], mybir.dt.float32, tag=f"xt{i}")
                    load_eng.dma_start(out=xt[:], in_=sl(x, a, w))
                    mt = pool.tile([P, w], mybir.dt.uint8, tag=f"mt{i}")
                    load_eng.dma_start(out=mt[:], in_=sl(m, a, w))
                    xts.append(xt)
                    mts.append(mt)
                for i in range(len(sizes)):
                    # mask is pre-scaled to {0,2}; one in-place op per tile
                    nc.vector.tensor_mul(
                        out=xts[i][:], in0=xts[i][:], in1=mts[i][:])
                order = reversed(range(len(sizes))) if rev_store \
                    else range(len(sizes))
                for i in order:
                    store_eng = nc.scalar if i % 2 == 0 else nc.sync
                    store_eng.dma_start(
                        out=sl(o, offs[i], sizes[i]), in_=xts[i][:])
    nc.compile()
    return nc


def _build_runner(nc, n_cores):
    """Compile the SPMD module into a reusable shard_map-jitted callable.

    Same machinery as bass2jax.run_bass_via_pjrt, but the jitted function is
    built once and cached so repeated kernel() calls skip XLA re-tracing.
    Output-buffer donation is dropped: this kernel writes every output
    element, so zero-initialized outputs are unnecessary.
    """
    import jax
    from jax.sharding import Mesh, PartitionSpec, NamedSharding
    from jax.experimental.shard_map import shard_map
    from concourse.bass2jax import (
        _bass_exec_p,
        install_neuronx_cc_hook,
        partition_id_tensor,
    )

    install_neuronx_cc_hook()
    partition_name = nc.partition_id_tensor.name if nc.partition_id_tensor else None

    in_names, out_names, out_avals = [], [], []
    for alloc in nc.m.functions[0].allocations:
        if not isinstance(alloc, mybir.MemoryLocationSet):
            continue
        name = alloc.memorylocations[0].name
        if alloc.kind == "ExternalInput":
            if name != partition_name:
                in_names.append(name)
        elif alloc.kind == "ExternalOutput":
            out_names.append(name)
            out_avals.append(
                jax.core.ShapedArray(
                    tuple(alloc.tensor_shape), mybir.dt.np(alloc.dtype)
                )
            )
    n_params = len(in_names)
    all_in_names = list(in_names) + list(out_names)
    if partition_name is not None:
        all_in_names.append(partition_name)

    def _body(*args):
        operands = list(args)
        if partition_name is not None:
            operands.append(partition_id_tensor())
        outs = _bass_exec_p.bind(
            *operands,
            out_avals=tuple(out_avals),
            in_names=tuple(all_in_names),
            out_names=tuple(out_names),
            lowering_input_output_aliases=(),
            sim_require_finite=True,
            sim_require_nnan=True,
            nc=nc,
        )
        return tuple(outs)

    devices = jax.devices()[:n_cores]
    assert len(devices) == n_cores, (
        f"need {n_cores} devices, have {len(jax.devices())}"
    )
    mesh = Mesh(np.asarray(devices), ("core",))
    in_specs = (PartitionSpec("core"),) * (n_params + len(out_names))
    out_specs = (PartitionSpec("core"),) * len(out_names)
    fn = jax.jit(
        shard_map(
            _body, mesh=mesh, in_specs=in_specs, out_specs=out_specs,
            check_rep=False,
        ),
        keep_unused=True,
    )
    sharding = NamedSharding(mesh, PartitionSpec("core"))
    zeros = [
        np.zeros((n_cores * a.shape[0], *a.shape[1:]), a.dtype) for a in out_avals
    ]
    return fn, sharding, in_names, out_avals, zeros


_CACHE = {}


def _get_runner():
    if "runner" not in _CACHE:
        nc = build_nc()
        _CACHE["runner"] = _build_runner(nc, N_CORES)
    return _CACHE["runner"]


def kernel(x: np.ndarray, mask: np.ndarray, **_) -> np.ndarray:
    import jax

    x = np.ascontiguousarray(np.asarray(x), dtype=np.float32)
    mask = np.asarray(mask)
    if mask.dtype.itemsize != 1:
        mask = mask.astype(np.bool_)
    mask = np.ascontiguousarray(mask)
    assert x.shape == (B, C, FM, H, W), x.shape
    assert mask.shape == (B, C, FM, H, W), mask.shape

    fn, sharding, in_names, out_avals, zeros = _get_runner()
    # Flat layout: batch-sharding == contiguous row-blocks, and the kernel's
    # element order is plain C order, so operands are zero-copy reshapes.
    # {0,1} bool bytes -> {0,2} u8 folds the dropout scale into the mask.
    global_in = {
        "x": x.reshape(N_CORES * ELEMS_PER_CORE),
        "mask": (mask.view(np.uint8) << 1).reshape(N_CORES * ELEMS_PER_CORE),
    }
    if "zeros_dev" not in _CACHE:
        # Output buffers are fully overwritten by the kernel; stage once and
        # reuse across calls (not donated).
        _CACHE["zeros_dev"] = [jax.device_put(z, sharding) for z in zeros]
    args = [jax.device_put(global_in[n], sharding) for n in in_names]
    args += _CACHE["zeros_dev"]
    out = jax.block_until_ready(fn(*args))
    return np.asarray(out[0]).reshape(B, C, FM, H, W)


# revision 17
# speedup vs baseline: 1.0701x; 1.0115x over previous
"""Inverted-dropout kernel for Trainium2, distributed over 8 NeuronCores.

Computes out = where(mask, x * 2.0, 0) for x:(64,2048,4,7,7) f32 and
mask:(64,2048,4,7,7) bool.  Pure elementwise: shard along batch (8 per core).

Design (each refinement HW-measured):
- Host folds the 1/(1-p)=2.0 dropout scale into the byte mask ({0,1} bool ->
  {0,2} u8), so the device does ONE DVE TensorTensor op per tile, reading the
  mask directly as uint8 — no cast op, mask HBM traffic stays 1 B/elem.
- Phase structure: ALL loads enqueue first, then in-place DVE muls, then ALL
  stores.  HWDGE rings drain FIFO per issuing engine, so reads and writes
  phase-separate at the HBM; measured pure-read ~457 GB/s and pure-write
  ~554 GB/s vs only ~430 GB/s for packet-interleaved mixed traffic.
- Ramp tiling (6x3584 + 4x896): small tiles last shrink the exposed
  final-tile DVE latency between the load and store phases (~3.8us -> ~1us).
- Mask consolidation: the mask loads as 2 large transfers (1.79 + 1.34 MB,
  group boundaries aligned to logical tiles) instead of 10 small ones —
  small mask transfers were descriptor-dominated and dragged the load phase.
- 2D [128, 25088] row-major per-core layout: column ranges are consistent
  for every tensor regardless of transfer grouping; host shard/gather are
  zero-copy reshapes.
- Whole per-core shard stays SBUF-resident (in-place DVE output => ~125 KB
  of the 192 KB per partition).
- Loads/stores alternate between the two HWDGE rings (SP / ACT).

Measured ~58-63 us per core quiet-window (~28.9 MB HBM traffic per core,
~460-500 GB/s aggregate); ~100 us under neighbor-tenant HBM contention.
"""

import sys

import numpy as np

try:
    import concourse.bacc as bacc
except ImportError:  # grading env without the default sys.path site config
    for p in ("/root/.axon_site/_ro/trn_rl_repo", "/opt/trn_rl_repo"):
        if p not in sys.path:
            sys.path.append(p)
    import concourse.bacc as bacc

import concourse.mybir as mybir
from concourse.tile import TileContext

# Full problem shape (hardcoded per harness contract).
B, C, FM, H, W = 64, 2048, 4, 7, 7
N_CORES = 8
B_PER_CORE = B // N_CORES                       # 8
ELEMS_PER_CORE = B_PER_CORE * C * FM * H * W    # 3,211,264 = 128 * 25088

P = 128                                         # SBUF partitions
TOTAL_F = ELEMS_PER_CORE // P                   # 25088 free-dim elems/partition
SIZES = [3584] * 6 + [896] * 4                  # ramp: small tiles last
MASK_GROUPS = [4, 6]                            # mask tiles: tiles 0-3, 4-9
assert sum(SIZES) == TOTAL_F

SCALE = 2.0      # 1 / (1 - p_drop), p_drop = 0.5


def build_nc(sizes=None, mask_groups=None, repeat=1, rev_store=False):
    """Build the per-core SPMD module (phase-structured, ramp-tiled,
    mask-consolidated).

    Bacc (not bare Bass): Bacc.compile() legalizes sync waits down to the
    TRN2 1-wait-per-instruction limit — walrus rejects the module otherwise.

    repeat>1 unrolls the whole body R times inside one NEFF (idempotent
    rewrites of the same output), used only for launch-overhead-free timing
    via (T(R2)-T(R1))/(R2-R1).  rev_store reverses per-repeat store order so
    cross-repeat WAR chains approximate clean serial load/store phases
    (timing only; production single-shot uses forward order).
    """
    sizes = sizes or SIZES
    mask_groups = mask_groups or MASK_GROUPS
    total = sum(sizes)
    nc = bacc.Bacc()
    x = nc.declare_dram_parameter("x", [P, total], mybir.dt.float32, isOutput=False)
    m = nc.declare_dram_parameter("mask", [P, total], mybir.dt.uint8, isOutput=False)
    o = nc.declare_dram_parameter("out", [P, total], mybir.dt.float32, isOutput=True)
    offs = np.cumsum([0] + list(sizes))[:-1]
    gb = np.cumsum([0] + list(mask_groups))
    granges = [
        (offs[gb[k]], (offs[gb[k + 1] - 1] + sizes[gb[k + 1] - 1]) - offs[gb[k]])
        for k in range(len(mask_groups))
    ]
    tile2group = {}
    for k in range(len(mask_groups)):
        for t in range(gb[k], gb[k + 1]):
            tile2group[t] = k

    with TileContext(nc) as tc:
        with tc.tile_pool(name="sbuf", bufs=1) as pool:
            for _ in range(repeat):
                mtiles = []
                for k, (ga, gw) in enumerate(granges):
                    eng = nc.sync if k % 2 == 0 else nc.scalar
                    mt = pool.tile([P, gw], mybir.dt.uint8, tag=f"mt{k}")
                    eng.dma_start(out=mt[:], in_=m[:, ga:ga + gw])
                    mtiles.append((mt, ga))
                xts = []
                for i, (a, w) in enumerate(zip(offs, sizes)):
                    load_eng = nc.sync if i % 2 == 0 else nc.scalar
                    xt = pool.tile([P, w], mybir.dt.float32, tag=f"xt{i}")
                    load_eng.dma_start(out=xt[:], in_=x[:, a:a + w])
                    xts.append(xt)
                for i, (a, w) in enumerate(zip(offs, sizes)):
                    # mask is pre-scaled to {0,2}; one in-place op per tile
                    mt, ga = mtiles[tile2group[i]]
                    nc.vector.tensor_mul(
                        out=xts[i][:], in0=xts[i][:],
                        in1=mt[:, a - ga: a - ga + w])
                order = reversed(range(len(sizes))) if rev_store \
                    else range(len(sizes))
                for i in order:
                    store_eng = nc.scalar if i % 2 == 0 else nc.sync
                    store_eng.dma_start(
                        out=o[:, offs[i]:offs[i] + sizes[i]], in_=xts[i][:])
    nc.compile()
    return nc


def _build_runner(nc, n_cores):
    """Compile the SPMD module into a reusable shard_map-jitted callable.

    Same machinery as bass2jax.run_bass_via_pjrt, but the jitted function is
    built once and cached so repeated kernel() calls skip XLA re-tracing.
    Output-buffer donation is dropped: this kernel writes every output
    element, so zero-initialized outputs are unnecessary.
    """
    import jax
    from jax.sharding import Mesh, PartitionSpec, NamedSharding
    from jax.experimental.shard_map import shard_map
    from concourse.bass2jax import (
        _bass_exec_p,
        install_neuronx_cc_hook,
        partition_id_tensor,
    )

    install_neuronx_cc_hook()
    partition_name = nc.partition_id_tensor.name if nc.partition_id_tensor else None

    in_names, out_names, out_avals = [], [], []
    for alloc in nc.m.functions[0].allocations:
        if not isinstance(alloc, mybir.MemoryLocationSet):
            continue
        name = alloc.memorylocations[0].name
        if alloc.kind == "ExternalInput":
            if name != partition_name:
                in_names.append(name)
        elif alloc.kind == "ExternalOutput":
            out_names.append(name)
            out_avals.append(
                jax.core.ShapedArray(
                    tuple(alloc.tensor_shape), mybir.dt.np(alloc.dtype)
                )
            )
    n_params = len(in_names)
    all_in_names = list(in_names) + list(out_names)
    if partition_name is not None:
        all_in_names.append(partition_name)

    def _body(*args):
        operands = list(args)
        if partition_name is not None:
            operands.append(partition_id_tensor())
        outs = _bass_exec_p.bind(
            *operands,
            out_avals=tuple(out_avals),
            in_names=tuple(all_in_names),
            out_names=tuple(out_names),
            lowering_input_output_aliases=(),
            sim_require_finite=True,
            sim_require_nnan=True,
            nc=nc,
        )
        return tuple(outs)

    devices = jax.devices()[:n_cores]
    assert len(devices) == n_cores, (
        f"need {n_cores} devices, have {len(jax.devices())}"
    )
    mesh = Mesh(np.asarray(devices), ("core",))
    in_specs = (PartitionSpec("core"),) * (n_params + len(out_names))
    out_specs = (PartitionSpec("core"),) * len(out_names)
    fn = jax.jit(
        shard_map(
            _body, mesh=mesh, in_specs=in_specs, out_specs=out_specs,
            check_rep=False,
        ),
        keep_unused=True,
    )
    sharding = NamedSharding(mesh, PartitionSpec("core"))
    zeros = [
        np.zeros((n_cores * a.shape[0], *a.shape[1:]), a.dtype) for a in out_avals
    ]
    return fn, sharding, in_names, out_avals, zeros


_CACHE = {}


def _get_runner():
    if "runner" not in _CACHE:
        nc = build_nc()
        _CACHE["runner"] = _build_runner(nc, N_CORES)
    return _CACHE["runner"]


def kernel(x: np.ndarray, mask: np.ndarray, **_) -> np.ndarray:
    import jax

    x = np.ascontiguousarray(np.asarray(x), dtype=np.float32)
    mask = np.asarray(mask)
    if mask.dtype.itemsize != 1:
        mask = mask.astype(np.bool_)
    mask = np.ascontiguousarray(mask)
    assert x.shape == (B, C, FM, H, W), x.shape
    assert mask.shape == (B, C, FM, H, W), mask.shape

    fn, sharding, in_names, out_avals, zeros = _get_runner()
    # 2D row-major layout: batch-sharding == row-blocks, so global operands
    # are zero-copy reshapes of the full inputs.  {0,1} bool bytes -> {0,2}
    # u8 folds the dropout scale into the mask (one cheap byte-op pass).
    global_in = {
        "x": x.reshape(N_CORES * P, TOTAL_F),
        "mask": (mask.view(np.uint8) << 1).reshape(N_CORES * P, TOTAL_F),
    }
    if "zeros_dev" not in _CACHE:
        # Output buffers are fully overwritten by the kernel; stage once and
        # reuse across calls (not donated).
        _CACHE["zeros_dev"] = [jax.device_put(z, sharding) for z in zeros]
    args = [jax.device_put(global_in[n], sharding) for n in in_names]
    args += _CACHE["zeros_dev"]
    out = jax.block_until_ready(fn(*args))
    return np.asarray(out[0]).reshape(B, C, FM, H, W)
